# revision 1
# baseline (speedup 1.0000x reference)
"""Chamfer loss kernel for TRN2 (8 NeuronCores, data-parallel over batch).

Reference computation (per batch b):
  t = l2_normalize(tokens[b])      # (K=1024, D=128)
  i = l2_normalize(interests[b])   # (M=64,  D=128)
  dist[k,m] = sqrt(2 - 2*dot(t_k, i_m))   (since ||t||=||i||=1)
  loss = mean_bm(min_k dist) + 0.3 * mean_bk(min_m dist)

min dist <=> max dot: reduce max over normalized dots, apply sqrt(2-2x)
only to tiny reduced tensors.

Structure (per core, 64 batches):
  phase 0: bulk-normalize+transpose ALL interests -> persistent iT_all
  per batch:
    DMA  tokens[b] -> t_all [128,(8,128)]
    PE   8 transposes (raw) -> psum -> copies -> tT [128d,(8n,128k)]
    PE   8 dots matmuls: pdots[k, (n m)] = tT.T @ iT_b
    ACT  tT2 = tT^2 (one big op)
    POOL partition-add over d -> sumsq per token (free-indexed row)
    DMA  4KB layout-gather: row -> tsum [128,8] (partition-indexed)
    ACT  sqrt, DVE reciprocal -> invt [128,8]
    DVE  dn = pdots * invt (fused normalize + psum evacuation)
    DVE  max over m -> staged [128,8]; max over n -> nmax [128,64]
    POOL partition-max over token partitions -> staged per-interest max
    every 4 batches: ACT sqrt(2-2x) on staged maxes, POOL accumulate
Host combines the 8 per-core partial sums.
"""

import os
import numpy as np
from contextlib import ExitStack

import concourse.bass as bass
import concourse.bass_isa as bass_isa
import concourse.mybir as mybir
import concourse.tile as tile
from concourse import bacc
from concourse.bass_utils import run_bass_kernel_spmd

N_CORES = 8
B, K, M, D = 512, 1024, 64, 128
B_LOC = B // N_CORES          # 64 batches per core
KT = K // 128                 # 8 token tiles of [128, D] per batch
ALPHA_T_TO_I = 0.3
STG = 4                       # sqrt-staging factor (batches per sqrt op)

F32 = mybir.dt.float32
AX = mybir.AxisListType
OP = mybir.AluOpType
ACT = mybir.ActivationFunctionType
RED = bass_isa.ReduceOp


def build(b_loc=B_LOC, reps=1):
    assert b_loc % 2 == 0 and b_loc % STG == 0
    nc = bacc.Bacc(
        "TRN2",
        target_bir_lowering=False,
        debug=False,
        num_devices=N_CORES,
    )
    tokens = nc.dram_tensor("tokens", [b_loc, K, D], F32, kind="ExternalInput").ap()
    interests = nc.dram_tensor(
        "interests", [b_loc, M, D], F32, kind="ExternalInput"
    ).ap()
    out = nc.dram_tensor("out", [1, 2], F32, kind="ExternalOutput").ap()

    NG = b_loc * M // 128     # interest row-groups of 128 (b_loc/2)

    with ExitStack() as ctx:
        tc = ctx.enter_context(tile.TileContext(nc))
        singles = ctx.enter_context(tc.tile_pool(name="singles", bufs=1))
        tok_pool = ctx.enter_context(tc.tile_pool(name="tok", bufs=5))
        work = ctx.enter_context(tc.tile_pool(name="work", bufs=5))
        small = ctx.enter_context(tc.tile_pool(name="small", bufs=10))
        stage = ctx.enter_context(tc.tile_pool(name="stage", bufs=4))
        srp = ctx.enter_context(tc.tile_pool(name="srp", bufs=4))
        p_tT = ctx.enter_context(tc.tile_pool(name="p_tT", bufs=3, space="PSUM"))
        p_dots = ctx.enter_context(tc.tile_pool(name="p_dots", bufs=5, space="PSUM"))
        dram = ctx.enter_context(tc.tile_pool(name="dram", bufs=8, space="DRAM"))

        identity = singles.tile([128, 128], F32)
        nc.gpsimd.memset(identity, 0.0)
        nc.gpsimd.affine_select(
            out=identity, in_=identity, compare_op=OP.not_equal, fill=1.0,
            base=0, pattern=[[-1, 128]], channel_multiplier=1,
        )
        ones = singles.tile([128, 1], F32)
        nc.vector.memset(ones, 1.0)
        two = singles.tile([128, 1], F32)
        nc.vector.memset(two, 2.0)
        acc_t = singles.tile([128, STG * KT], F32)
        acc_i = singles.tile([1, STG * M], F32)
        nc.vector.memset(acc_t, 0.0)
        nc.vector.memset(acc_i, 0.0)

        # ---------- phase 0: all interests -> normalized iT_all ----------
        # interests flat (b*M, D) -> groups of 128 rows
        i_flat = interests.rearrange("b m d -> (b m) d").rearrange(
            "(g p) d -> p g d", p=128
        )  # [128, NG, 128]
        i_all = singles.tile([128, NG, D], F32)
        nc.sync.dma_start(out=i_all, in_=i_flat)
        isum = singles.tile([128, NG], F32)
        itrash = work.tile([128, D], F32, tag="trash")
        for g in range(NG):
            nc.scalar.activation(
                itrash, i_all[:, g, :], ACT.Square, accum_out=isum[:, g:g + 1]
            )
        inrm = singles.tile([128, NG], F32)
        nc.scalar.sqrt(inrm, isum)
        invi = singles.tile([128, NG], F32)
        nc.vector.reciprocal(invi, inrm)
        nc.vector.tensor_mul(i_all, i_all, invi.broadcast_to([128, NG, D]))
        iT_all = singles.tile([128, NG, 128], F32)   # [d, (g, bm)]
        for h0 in range(0, NG, 4):
            cn = min(4, NG - h0)
            piT = p_tT.tile([128, 512], F32, tag="ptT")
            for j in range(cn):
                g = h0 + j
                nc.tensor.transpose(
                    piT[:, 128 * j:128 * (j + 1)], i_all[:, g, :], identity
                )
            dst = iT_all[:, h0:h0 + cn, :].rearrange("p a b -> p (a b)")
            if (h0 // 4) % 2 == 0:
                nc.vector.tensor_copy(dst, piT[:, :128 * cn])
            else:
                nc.scalar.copy(dst, piT[:, :128 * cn])

        def iT_of(b):
            # batch b's interests: rows (b*M..b*M+M) = group b//2, half b%2
            return iT_all[:, b // 2, (b % 2) * M:(b % 2) * M + M]

        # ---------- main loop (software-pipelined: tail lags front by STG) ----------
        pdots_of = {}
        tsum_of = {}
        invt_of = {}
        st_of = {}
        LAG = 4

        def front(vb):
            b = vb % b_loc
            t_all = tok_pool.tile([128, KT, D], F32)
            nc.sync.dma_start(
                out=t_all, in_=tokens[b].rearrange("(n p) d -> p n d", p=128)
            )

            # transposes of raw token tiles
            tT = work.tile([128, KT, 128], F32, tag="tT")
            for h in range(2):
                ptT = p_tT.tile([128, 512], F32)
                for j in range(4):
                    n = 4 * h + j
                    nc.tensor.transpose(
                        ptT[:, 128 * j:128 * (j + 1)], t_all[:, n, :], identity
                    )
                dst = tT[:, 4 * h:4 * (h + 1), :].rearrange("p a b -> p (a b)")
                if h == 0:
                    nc.vector.tensor_copy(dst, ptT)
                else:
                    nc.scalar.copy(dst, ptT)

            # sum of squares over d via ACT square + POOL partition-add
            tT2 = work.tile([128, KT, 128], F32, tag="tT2")
            nc.scalar.square(tT2, tT)
            s_rep = srp.tile([128, KT, 128], F32, tag="s_rep")
            nc.gpsimd.partition_all_reduce(
                s_rep.rearrange("p a b -> p (a b)"),
                tT2.rearrange("p a b -> p (a b)"),
                channels=128, reduce_op=RED.add,
            )
            # layout gather row [1,(n,k)] -> [128(k), n] via DRAM bounce,
            # on the scalar/vector DMA queues to keep the sync queue free
            scr = dram.tile([1, K], F32, tag="scr")
            nc.scalar.dma_start(
                out=scr, in_=s_rep[0:1].rearrange("p a b -> p (a b)")
            )
            tsum = small.tile([128, KT], F32, tag="tsum")
            nc.scalar.dma_start(
                out=tsum, in_=scr.rearrange("o (n p) -> (o p) n", p=128)
            )
            tsum_of[vb] = tsum

            # dots (raw tokens x normalized interests)
            pdots = p_dots.tile([128, KT, M], F32)
            iT = iT_of(b)  # noqa: uses real batch index
            for n in range(KT):
                nc.tensor.matmul(
                    pdots[:, n, :], lhsT=tT[:, n, :], rhs=iT,
                    start=True, stop=True,
                )
            pdots_of[vb] = pdots

        def tail(bb):
            s2 = bb % STG
            g = bb // STG
            if s2 == 0:
                st_t_new = stage.tile([128, STG, KT], F32, tag="st_t")
                st_i_new = stage.tile([128, STG, M], F32, tag="st_i")
                st_of[g] = (st_t_new, st_i_new)
            st_t, st_i = st_of[g]
            pd = pdots_of.pop(bb)
            tnrm = small.tile([128, KT], F32, tag="tnrm")
            nc.scalar.sqrt(tnrm, tsum_of.pop(bb))
            invt = small.tile([128, KT], F32, tag="invt")
            nc.vector.reciprocal(invt, tnrm)
            # fused normalize + evacuate
            dn = work.tile([128, KT, M], F32, tag="dn")
            nc.vector.tensor_mul(
                dn, pd, invt.broadcast_to([128, KT, M])
            )
            # per-token max over m
            nc.vector.tensor_reduce(st_t[:, s2, :], dn, axis=AX.X, op=OP.max)
            # per-interest: max over n (DVE), then partitions (POOL)
            nmax = small.tile([128, M], F32, tag="nmax")
            nc.vector.tensor_reduce(
                nmax, dn.rearrange("p n m -> p m n"), axis=AX.X, op=OP.max
            )
            nc.gpsimd.partition_all_reduce(
                st_i[:, s2, :], nmax, channels=128, reduce_op=RED.max
            )
            if s2 == STG - 1:
                del st_of[g]
                dts = stage.tile([128, STG * KT], F32, tag="dts")
                nc.scalar.activation(
                    dts, st_t.rearrange("p a b -> p (a b)"),
                    ACT.Sqrt, bias=two[:], scale=-2.0,
                )
                nc.gpsimd.tensor_add(acc_t, acc_t, dts)
                dis = stage.tile([1, STG * M], F32, tag="dis")
                nc.scalar.activation(
                    dis, st_i[0:1].rearrange("o a b -> o (a b)"),
                    ACT.Sqrt, bias=two[:1], scale=-2.0,
                )
                nc.gpsimd.tensor_add(acc_i, acc_i, dis)

        nvb = b_loc * reps
        for vb in range(nvb + LAG):
            if vb < nvb:
                front(vb)
            if vb >= LAG:
                tail(vb - LAG)

        # ---------- final reductions ----------
        red_t = singles.tile([128, 1], F32)
        nc.vector.tensor_reduce(red_t, acc_t, axis=AX.X, op=OP.add)
        pfin = p_dots.tile([128, M], F32, tag="pdots")
        nc.tensor.matmul(pfin[:1, :1], lhsT=ones, rhs=red_t, start=True, stop=True)
        out_sb = small.tile([1, 2], F32, tag="out_sb")
        nc.scalar.copy(out_sb[:, 0:1], pfin[:1, :1])
        nc.vector.tensor_reduce(out_sb[:, 1:2], acc_i, axis=AX.X, op=OP.add)
        nc.sync.dma_start(out=out, in_=out_sb)

    nc.compile()
    return nc


_NC_CACHE = None


def _get_nc():
    global _NC_CACHE
    if _NC_CACHE is None:
        _NC_CACHE = build()
    return _NC_CACHE


def kernel(tokens: np.ndarray, interests: np.ndarray, _trace=False) -> np.ndarray:
    tokens = np.ascontiguousarray(tokens, dtype=np.float32)
    interests = np.ascontiguousarray(interests, dtype=np.float32)
    assert tokens.shape == (B, K, D) and interests.shape == (B, M, D)

    nc = _get_nc()
    in_maps = [
        {
            "tokens": tokens[c * B_LOC:(c + 1) * B_LOC],
            "interests": interests[c * B_LOC:(c + 1) * B_LOC],
        }
        for c in range(N_CORES)
    ]
    res = run_bass_kernel_spmd(
        nc, in_maps, core_ids=list(range(N_CORES)), trace=_trace
    )
    sum_t = 0.0  # sum over all (b, k) of min_m dist
    sum_i = 0.0  # sum over all (b, m) of min_k dist
    for r in res.results:
        sum_t += float(r["out"][0, 0])
        sum_i += float(r["out"][0, 1])
    loss = sum_i / (B * M) + ALPHA_T_TO_I * sum_t / (B * K)
    kernel.last_results = res
    return np.array(loss, dtype=np.float32)



# revision 11
# speedup vs baseline: 1.7611x; 1.7611x over previous
"""Chamfer loss kernel for TRN2 (8 NeuronCores, data-parallel over batch).

Reference computation (per batch b):
  t = l2_normalize(tokens[b])      # (K=1024, D=128)
  i = l2_normalize(interests[b])   # (M=64,  D=128)
  dist[k,m] = sqrt(2 - 2*dot(t_k, i_m))   (unit vectors)
  loss = mean_bm(min_k dist) + 0.3 * mean_bk(min_m dist)

Design notes (per core, 64 batches; engine-balanced against the ~1.46us/batch
token-DMA floor):
  - interests pre-normalized once (phase 0, bn_stats for sum-of-squares),
    kept transposed in SBUF as bf16 iT_all.
  - per batch:
      DMA  tokens[b] -> t_all fp32 [128,(8n),128d]
      PE   8 transposes (fp32) -> psum; ACT evacuates psum -> tT bf16
      DVE  tq = tT*tT (bf16 2x mode)
      PE   8 dot matmuls   pdots[k,(n m)] = tT_n.T @ iT_b        (bf16)
      PE   8 ones-column matmuls sums[k,n] = tq_n.T @ ones  == sum_d t^2
           (lands token sum-of-squares directly in [k-partition, n] layout:
            no partition reduce, no DMA gather)
      ACT  tnrm = sqrt(sums) from psum;  DVE invt = 1/tnrm
      DVE  dn = pdots * invt  (fused normalize + psum evacuation, bf16)
      POOL st_t_all[:,b,:] = max_m dn   (deferred sqrt, once at the end)
      POOL m2 = max(dn[:,0:4,:], dn[:,4:8,:]);  DVE m3, nmax (bf16 tree)
      POOL partition-max nmax -> st_i chunk;  every 8 batches ACT applies
           sqrt(2-2x) to the chunk and POOL accumulates.
Host combines the 8 per-core partial sums.
"""

import numpy as np
from contextlib import ExitStack

import concourse.bass as bass
import concourse.bass_isa as bass_isa
import concourse.mybir as mybir
import concourse.tile as tile
from concourse import bacc
from concourse.bass_utils import run_bass_kernel_spmd

N_CORES = 8
B, K, M, D = 512, 1024, 64, 128
B_LOC = B // N_CORES          # 64 batches per core
KT = K // 128                 # 8 token tiles of [128, D] per batch
NG = B_LOC * M // 128         # 32 interest row-groups of 128
ALPHA_T_TO_I = 0.3
SI = 8                        # i-side sqrt staging (batches per chunk)
LAG = 5

F32 = mybir.dt.float32
BF16 = mybir.dt.bfloat16
AX = mybir.AxisListType
OP = mybir.AluOpType
ACT = mybir.ActivationFunctionType
RED = bass_isa.ReduceOp


def build(b_loc=B_LOC):
    assert b_loc % SI == 0
    nc = bacc.Bacc(
        "TRN2",
        target_bir_lowering=False,
        debug=False,
        num_devices=N_CORES,
    )
    tokens = nc.dram_tensor("tokens", [b_loc, K, D], F32, kind="ExternalInput").ap()
    interests = nc.dram_tensor(
        "interests", [b_loc, M, D], F32, kind="ExternalInput"
    ).ap()
    out = nc.dram_tensor("out", [1, 2], F32, kind="ExternalOutput").ap()

    with ExitStack() as ctx:
        tc = ctx.enter_context(tile.TileContext(nc))
        singles = ctx.enter_context(tc.tile_pool(name="singles", bufs=1))
        tok_pool = ctx.enter_context(tc.tile_pool(name="tok", bufs=3))
        tT_pool = ctx.enter_context(tc.tile_pool(name="tT", bufs=3))
        tq_pool = ctx.enter_context(tc.tile_pool(name="tq", bufs=3))
        dn_pool = ctx.enter_context(tc.tile_pool(name="dn", bufs=3))
        m2_pool = ctx.enter_context(tc.tile_pool(name="m2", bufs=3))
        small = ctx.enter_context(tc.tile_pool(name="small", bufs=10))
        stage = ctx.enter_context(tc.tile_pool(name="stage", bufs=2))
        p_tT = ctx.enter_context(tc.tile_pool(name="p_tT", bufs=3, space="PSUM"))
        p_dots = ctx.enter_context(tc.tile_pool(name="p_dots", bufs=2, space="PSUM"))
        p_sums = ctx.enter_context(tc.tile_pool(name="p_sums", bufs=1, space="PSUM"))

        identity = singles.tile([128, 128], F32)
        nc.gpsimd.memset(identity, 0.0)
        nc.gpsimd.affine_select(
            out=identity, in_=identity, compare_op=OP.not_equal, fill=1.0,
            base=0, pattern=[[-1, 128]], channel_multiplier=1,
        )
        ones_bf = singles.tile([128, 1], BF16)
        nc.vector.memset(ones_bf, 1.0)
        two = singles.tile([128, 1], F32)
        nc.vector.memset(two, 2.0)
        st_t_all = singles.tile([128, b_loc, KT], BF16)
        acc_i = singles.tile([128, SI * M], F32)
        nc.vector.memset(acc_i, 0.0)

        # ---------- phase 0: normalize + transpose all interests ----------
        i_flat = interests.rearrange("b m d -> (b m) d").rearrange(
            "(g p) d -> p g d", p=128
        )  # [128, NG, 128]
        i_all = singles.tile([128, NG, D], F32)
        nc.sync.dma_start(out=i_all, in_=i_flat)

        # sum-of-squares per interest row: ACT square + DVE segmented reduce
        isq = singles.tile([128, NG, D], BF16)
        H2 = NG // 2
        nc.scalar.square(isq[:, :H2], i_all[:, :H2])
        nc.scalar.square(isq[:, H2:], i_all[:, H2:])
        issq = small.tile([128, NG], F32, tag="issq")
        nc.vector.tensor_reduce(issq, isq, axis=AX.X, op=OP.add)
        inrm = small.tile([128, NG], F32, tag="inrm")
        nc.scalar.sqrt(inrm, issq)
        invi = small.tile([128, NG], F32, tag="invi")
        nc.vector.reciprocal(invi, inrm)

        i_n = singles.tile([128, NG, D], F32)
        H = NG // 2
        nc.vector.tensor_mul(
            i_n[:, :H], i_all[:, :H], invi[:, :H].broadcast_to([128, H, D])
        )
        nc.gpsimd.tensor_mul(
            i_n[:, H:], i_all[:, H:], invi[:, H:].broadcast_to([128, H, D])
        )

        iT_all = singles.tile([128, NG, 128], BF16)  # [d, (g, row)]
        for c in range(0, NG, 4):
            piT = p_tT.tile([128, 4, 128], F32, tag="ptT")
            for j in range(4):
                nc.tensor.transpose(
                    piT[:, j, :], i_n[:, c + j, :], identity
                )
            dst = iT_all[:, c:c + 4, :].rearrange("p a b -> p (a b)")
            src = piT[:, :4, :].rearrange("p a b -> p (a b)")
            if c < 24:
                nc.scalar.copy(dst, src)
            else:
                nc.vector.tensor_copy(dst, src)

        def iT_of(b):
            return iT_all[:, b // 2, (b % 2) * M:(b % 2) * M + M]

        # ---------- software-pipelined main loop ----------
        # Post-matmul vector work is fused over batch PAIRS to amortize the
        # fixed per-op access latencies on DVE.  All free-axis reductions and
        # maxes are DVE-only (gpsimd has neither); Pool gets the elementwise
        # square's other half, the partition-max, and the accumulate adds.
        t_of, tT_of, pd_of, ps_of, iv_of = {}, {}, {}, {}, {}
        m2_of, sti_of = {}, {}

        def s0(b):  # token DMA
            t_all = tok_pool.tile([128, KT, D], F32)
            nc.sync.dma_start(
                out=t_all, in_=tokens[b].rearrange("(n p) d -> p n d", p=128)
            )
            t_of[b] = t_all

        def s1(b):  # transposes + evacuation (fp32 psum -> bf16 sbuf)
            t_all = t_of.pop(b)
            tT = tT_pool.tile([128, KT, 128], BF16, tag="tT")
            for h in range(2):
                ptT = p_tT.tile([128, KT // 2, 128], F32, tag="ptT")
                for j in range(KT // 2):
                    nc.tensor.transpose(
                        ptT[:, j, :], t_all[:, 4 * h + j, :], identity
                    )
                nc.scalar.copy(
                    tT[:, 4 * h:4 * h + 4, :].rearrange("p a b -> p (a b)"),
                    ptT.rearrange("p a b -> p (a b)"),
                )
            tT_of[b] = tT

        def s2(b):  # squares (DVE/Pool halves), dots, sum-of-squares columns
            tT = tT_of.pop(b)
            tq = tq_pool.tile([128, KT, 128], BF16, tag="tq")
            nc.vector.tensor_mul(
                tq[:, 0:4, :].rearrange("p a b -> p (a b)"),
                tT[:, 0:4, :].rearrange("p a b -> p (a b)"),
                tT[:, 0:4, :].rearrange("p a b -> p (a b)"),
            )
            nc.gpsimd.tensor_mul(
                tq[:, 4:8, :].rearrange("p a b -> p (a b)"),
                tT[:, 4:8, :].rearrange("p a b -> p (a b)"),
                tT[:, 4:8, :].rearrange("p a b -> p (a b)"),
            )
            if b % 2 == 0:
                pd2 = p_dots.tile([128, 2, KT, M], F32, tag="pd")
                ps2 = p_sums.tile([128, 2, KT], F32, tag="ps")
                pd_of[b // 2] = pd2
                ps_of[b // 2] = ps2
            else:
                pd2 = pd_of[b // 2]
                ps2 = ps_of[b // 2]
            h = b % 2
            iT = iT_of(b)
            for n in range(KT):
                nc.tensor.matmul(
                    pd2[:, h, n, :], lhsT=tT[:, n, :], rhs=iT,
                    start=True, stop=True,
                )
            for n in range(KT):
                nc.tensor.matmul(
                    ps2[:, h, n:n + 1], lhsT=tq[:, n, :], rhs=ones_bf,
                    start=True, stop=True,
                )

        def s3(j):  # token norms for pair j
            tnrm = small.tile([128, 2, KT], F32, tag="tnrm")
            nc.scalar.sqrt(tnrm, ps_of.pop(j))
            invt = small.tile([128, 2, KT], F32, tag="invt")
            nc.vector.reciprocal(invt, tnrm)
            iv_of[j] = invt

        def s4(j):  # normalize; max over m; first n-tree level  (pair j)
            pd2 = pd_of.pop(j)
            invt = iv_of.pop(j)
            dn = dn_pool.tile([128, 2, KT, M], BF16, tag="dn")
            nc.vector.tensor_mul(dn, pd2, invt.broadcast_to([128, 2, KT, M]))
            # t->i: per-token max over m -> full-run store (sqrt deferred)
            nc.vector.tensor_reduce(
                st_t_all[:, 2 * j:2 * j + 2, :], dn, axis=AX.X, op=OP.max
            )
            # i->t: max over n, tree level 1 (8 -> 4)
            m2 = m2_pool.tile([128, 2, KT // 2, M], BF16, tag="m2")
            nc.vector.tensor_max(
                m2, dn[:, :, 0:KT // 2, :], dn[:, :, KT // 2:KT, :]
            )
            m2_of[j] = m2

        def s5(j):  # finish i->t reduction; staged sqrt every SI batches
            b0 = 2 * j
            s2i = b0 % SI
            g = b0 // SI
            if s2i == 0:
                st_i_new = stage.tile([128, SI, M], BF16, tag="sti")
                sti_of[g] = st_i_new
            st_i = sti_of[g]
            m2 = m2_of.pop(j)
            m3 = small.tile([128, 2, 2, M], BF16, tag="m3")
            nc.vector.tensor_max(m3, m2[:, :, 0:2, :], m2[:, :, 2:4, :])
            nm2 = small.tile([128, 2, M], BF16, tag="nm2")
            nc.vector.tensor_max(nm2, m3[:, :, 0, :], m3[:, :, 1, :])
            nc.gpsimd.partition_all_reduce(
                st_i[:, s2i:s2i + 2, :].rearrange("p a b -> p (a b)"),
                nm2.rearrange("p a b -> p (a b)"),
                channels=128, reduce_op=RED.max,
            )
            if s2i == SI - 2:
                del sti_of[g]
                di = stage.tile([128, SI * M], BF16, tag="di")
                nc.scalar.activation(
                    di, st_i.rearrange("p a b -> p (a b)"),
                    ACT.Sqrt, bias=two[:], scale=-2.0,
                )
                nc.gpsimd.tensor_add(acc_i, acc_i, di)

        nj = b_loc // 2
        for v in range(b_loc + 2 * LAG):
            # pair stages run at half rate, interleaved with batch stages
            if v >= LAG and (v - LAG) % 2 == 1 and (v - LAG) // 2 < nj:
                s5((v - LAG) // 2)
            if v >= 4 and (v - 4) % 2 == 1 and (v - 4) // 2 < nj:
                s4((v - 4) // 2)
            if v >= 3 and (v - 3) % 2 == 1 and (v - 3) // 2 < nj:
                s3((v - 3) // 2)
            if v >= 2 and v - 2 < b_loc:
                s2(v - 2)
            if v >= 1 and v - 1 < b_loc:
                s1(v - 1)
            if v < b_loc:
                s0(v)

        # ---------- final reductions ----------
        dt = singles.tile([128, b_loc * KT], BF16)
        nc.scalar.activation(
            dt, st_t_all.rearrange("p a b -> p (a b)"),
            ACT.Sqrt, bias=two[:], scale=-2.0,
        )
        red_t = singles.tile([128, 1], F32)
        nc.vector.tensor_reduce(red_t, dt, axis=AX.X, op=OP.add)
        rep_t = singles.tile([128, 1], F32)
        nc.gpsimd.partition_all_reduce(
            rep_t, red_t, channels=128, reduce_op=RED.add
        )
        red_i = singles.tile([128, 1], F32)
        nc.vector.tensor_reduce(red_i, acc_i, axis=AX.X, op=OP.add)
        out_sb = small.tile([1, 2], F32, tag="out_sb")
        nc.scalar.copy(out_sb[:, 0:1], rep_t[0:1, :])
        nc.scalar.copy(out_sb[:, 1:2], red_i[0:1, :])
        nc.sync.dma_start(out=out, in_=out_sb)

    nc.compile()
    return nc


_NC_CACHE = None


def _get_nc():
    global _NC_CACHE
    if _NC_CACHE is None:
        _NC_CACHE = build()
    return _NC_CACHE


def kernel(tokens: np.ndarray, interests: np.ndarray, _trace=False) -> np.ndarray:
    tokens = np.ascontiguousarray(tokens, dtype=np.float32)
    interests = np.ascontiguousarray(interests, dtype=np.float32)
    assert tokens.shape == (B, K, D) and interests.shape == (B, M, D)

    nc = _get_nc()
    in_maps = [
        {
            "tokens": tokens[c * B_LOC:(c + 1) * B_LOC],
            "interests": interests[c * B_LOC:(c + 1) * B_LOC],
        }
        for c in range(N_CORES)
    ]
    res = run_bass_kernel_spmd(
        nc, in_maps, core_ids=list(range(N_CORES)), trace=_trace
    )
    sum_t = 0.0  # sum over all (b, k) of min_m dist
    sum_i = 0.0  # sum over all (b, m) of min_k dist
    for r in res.results:
        sum_t += float(r["out"][0, 0])
        sum_i += float(r["out"][0, 1])
    loss = sum_i / (B * M) + ALPHA_T_TO_I * sum_t / (B * K)
    kernel.last_results = res
    return np.array(loss, dtype=np.float32)


# revision 13
# speedup vs baseline: 1.9304x; 1.0961x over previous
"""Chamfer loss kernel for TRN2 (8 NeuronCores, data-parallel over batch).

Reference computation (per batch b):
  t = l2_normalize(tokens[b])      # (K=1024, D=128)
  i = l2_normalize(interests[b])   # (M=64,  D=128)
  dist[k,m] = sqrt(2 - 2*dot(t_k, i_m))   (unit vectors)
  loss = mean_bm(min_k dist) + 0.3 * mean_bk(min_m dist)

Design notes (per core, 64 batches; engine-balanced against the ~1.46us/batch
token-DMA floor):
  - interests pre-normalized once (phase 0, bn_stats for sum-of-squares),
    kept transposed in SBUF as bf16 iT_all.
  - per batch:
      DMA  tokens[b] -> t_all fp32 [128,(8n),128d]
      PE   8 transposes (fp32) -> psum; ACT evacuates psum -> tT bf16
      DVE  tq = tT*tT (bf16 2x mode)
      PE   8 dot matmuls   pdots[k,(n m)] = tT_n.T @ iT_b        (bf16)
      PE   8 ones-column matmuls sums[k,n] = tq_n.T @ ones  == sum_d t^2
           (lands token sum-of-squares directly in [k-partition, n] layout:
            no partition reduce, no DMA gather)
      ACT  tnrm = sqrt(sums) from psum;  DVE invt = 1/tnrm
      DVE  dn = pdots * invt  (fused normalize + psum evacuation, bf16)
      POOL st_t_all[:,b,:] = max_m dn   (deferred sqrt, once at the end)
      POOL m2 = max(dn[:,0:4,:], dn[:,4:8,:]);  DVE m3, nmax (bf16 tree)
      POOL partition-max nmax -> st_i chunk;  every 8 batches ACT applies
           sqrt(2-2x) to the chunk and POOL accumulates.
Host combines the 8 per-core partial sums.
"""

import numpy as np
from contextlib import ExitStack

import concourse.bass as bass
import concourse.bass_isa as bass_isa
import concourse.mybir as mybir
import concourse.tile as tile
from concourse import bacc
from concourse.bass_utils import run_bass_kernel_spmd

N_CORES = 8
B, K, M, D = 512, 1024, 64, 128
B_LOC = B // N_CORES          # 64 batches per core
KT = K // 128                 # 8 token tiles of [128, D] per batch
NG = B_LOC * M // 128         # 32 interest row-groups of 128
ALPHA_T_TO_I = 0.3
SI = 8                        # i-side sqrt staging (batches per chunk)
LAG = 5

F32 = mybir.dt.float32
BF16 = mybir.dt.bfloat16
AX = mybir.AxisListType
OP = mybir.AluOpType
ACT = mybir.ActivationFunctionType
RED = bass_isa.ReduceOp


def build(b_loc=B_LOC):
    assert b_loc % SI == 0
    nc = bacc.Bacc(
        "TRN2",
        target_bir_lowering=False,
        debug=False,
        num_devices=N_CORES,
    )
    tokens = nc.dram_tensor("tokens", [b_loc, K, D], F32, kind="ExternalInput").ap()
    interests = nc.dram_tensor(
        "interests", [b_loc, M, D], F32, kind="ExternalInput"
    ).ap()
    out = nc.dram_tensor("out", [1, 2], F32, kind="ExternalOutput").ap()

    with ExitStack() as ctx:
        tc = ctx.enter_context(tile.TileContext(nc))
        singles = ctx.enter_context(tc.tile_pool(name="singles", bufs=1))
        tok_pool = ctx.enter_context(tc.tile_pool(name="tok", bufs=3))
        tT_pool = ctx.enter_context(tc.tile_pool(name="tT", bufs=3))
        tq_pool = ctx.enter_context(tc.tile_pool(name="tq", bufs=3))
        dn_pool = ctx.enter_context(tc.tile_pool(name="dn", bufs=3))
        m2_pool = ctx.enter_context(tc.tile_pool(name="m2", bufs=3))
        small = ctx.enter_context(tc.tile_pool(name="small", bufs=10))
        stage = ctx.enter_context(tc.tile_pool(name="stage", bufs=2))
        p_tT = ctx.enter_context(tc.tile_pool(name="p_tT", bufs=3, space="PSUM"))
        p_dots = ctx.enter_context(tc.tile_pool(name="p_dots", bufs=2, space="PSUM"))
        p_sums = ctx.enter_context(tc.tile_pool(name="p_sums", bufs=1, space="PSUM"))

        identity = singles.tile([128, 128], F32)
        nc.gpsimd.memset(identity, 0.0)
        nc.gpsimd.affine_select(
            out=identity, in_=identity, compare_op=OP.not_equal, fill=1.0,
            base=0, pattern=[[-1, 128]], channel_multiplier=1,
        )
        ones_bf = singles.tile([128, 1], BF16)
        nc.vector.memset(ones_bf, 1.0)
        two = singles.tile([128, 1], F32)
        nc.vector.memset(two, 2.0)
        st_t_all = singles.tile([128, b_loc, KT], BF16)
        acc_i = singles.tile([128, SI * M], F32)
        nc.vector.memset(acc_i, 0.0)

        # ---------- phase 0: normalize + transpose all interests ----------
        i_flat = interests.rearrange("b m d -> (b m) d").rearrange(
            "(g p) d -> p g d", p=128
        )  # [128, NG, 128]
        i_all = singles.tile([128, NG, D], F32)
        nc.sync.dma_start(out=i_all, in_=i_flat)

        # sum-of-squares per interest row: ACT square + DVE segmented reduce
        isq = singles.tile([128, NG, D], BF16)
        H2 = NG // 2
        nc.scalar.square(isq[:, :H2], i_all[:, :H2])
        nc.scalar.square(isq[:, H2:], i_all[:, H2:])
        issq = small.tile([128, NG], F32, tag="issq")
        nc.vector.tensor_reduce(issq, isq, axis=AX.X, op=OP.add)
        inrm = small.tile([128, NG], F32, tag="inrm")
        nc.scalar.sqrt(inrm, issq)
        invi = small.tile([128, NG], F32, tag="invi")
        nc.vector.reciprocal(invi, inrm)

        i_n = singles.tile([128, NG, D], F32)
        H = NG // 2
        nc.vector.tensor_mul(
            i_n[:, :H], i_all[:, :H], invi[:, :H].broadcast_to([128, H, D])
        )
        nc.gpsimd.tensor_mul(
            i_n[:, H:], i_all[:, H:], invi[:, H:].broadcast_to([128, H, D])
        )

        iT_all = singles.tile([128, NG, 128], BF16)  # [d, (g, row)]
        for c in range(0, NG, 4):
            piT = p_tT.tile([128, 4, 128], F32, tag="ptT")
            for j in range(4):
                nc.tensor.transpose(
                    piT[:, j, :], i_n[:, c + j, :], identity
                )
            dst = iT_all[:, c:c + 4, :].rearrange("p a b -> p (a b)")
            src = piT[:, :4, :].rearrange("p a b -> p (a b)")
            if c < 24:
                nc.scalar.copy(dst, src)
            else:
                nc.vector.tensor_copy(dst, src)

        def iT_of(b):
            return iT_all[:, b // 2, (b % 2) * M:(b % 2) * M + M]

        # ---------- software-pipelined main loop ----------
        # Post-matmul vector work is fused over batch PAIRS to amortize the
        # fixed per-op access latencies on DVE.  All free-axis reductions and
        # maxes are DVE-only (gpsimd has neither); Pool gets the elementwise
        # square's other half, the partition-max, and the accumulate adds.
        t_of, tT_of, pd_of, ps_of, iv_of = {}, {}, {}, {}, {}
        m2_of, sti_of = {}, {}

        def s0(b):  # token DMA
            t_all = tok_pool.tile([128, KT, D], F32)
            nc.sync.dma_start(
                out=t_all, in_=tokens[b].rearrange("(n p) d -> p n d", p=128)
            )
            t_of[b] = t_all

        def s1(b):  # transposes + evacuation (fp32 psum -> bf16 sbuf)
            t_all = t_of.pop(b)
            tT = tT_pool.tile([128, KT, 128], BF16, tag="tT")
            for h in range(2):
                ptT = p_tT.tile([128, KT // 2, 128], F32, tag="ptT")
                for j in range(KT // 2):
                    nc.tensor.transpose(
                        ptT[:, j, :], t_all[:, 4 * h + j, :], identity
                    )
                nc.scalar.copy(
                    tT[:, 4 * h:4 * h + 4, :].rearrange("p a b -> p (a b)"),
                    ptT.rearrange("p a b -> p (a b)"),
                )
            tT_of[b] = tT

        def s2(b):  # squares (DVE/Pool halves), dots, sum-of-squares columns
            tT = tT_of.pop(b)
            tq = tq_pool.tile([128, KT, 128], BF16, tag="tq")
            nc.vector.tensor_mul(
                tq[:, 0:4, :].rearrange("p a b -> p (a b)"),
                tT[:, 0:4, :].rearrange("p a b -> p (a b)"),
                tT[:, 0:4, :].rearrange("p a b -> p (a b)"),
            )
            nc.gpsimd.tensor_mul(
                tq[:, 4:8, :].rearrange("p a b -> p (a b)"),
                tT[:, 4:8, :].rearrange("p a b -> p (a b)"),
                tT[:, 4:8, :].rearrange("p a b -> p (a b)"),
            )
            if b % 2 == 0:
                pd2 = p_dots.tile([128, 2, KT, M], F32, tag="pd")
                ps2 = p_sums.tile([128, 2, KT], F32, tag="ps")
                pd_of[b // 2] = pd2
                ps_of[b // 2] = ps2
            else:
                pd2 = pd_of[b // 2]
                ps2 = ps_of[b // 2]
            h = b % 2
            iT = iT_of(b)
            for n in range(KT):
                nc.tensor.matmul(
                    pd2[:, h, n, :], lhsT=tT[:, n, :], rhs=iT,
                    start=True, stop=True,
                )
            for n in range(KT):
                nc.tensor.matmul(
                    ps2[:, h, n:n + 1], lhsT=tq[:, n, :], rhs=ones_bf,
                    start=True, stop=True,
                )

        def s3(j):  # token norms for pair j
            tnrm = small.tile([128, 2, KT], F32, tag="tnrm")
            nc.scalar.sqrt(tnrm, ps_of.pop(j))
            invt = small.tile([128, 2, KT], F32, tag="invt")
            nc.vector.reciprocal(invt, tnrm)
            iv_of[j] = invt

        def s4(j):  # normalize; max over m; first n-tree level  (pair j)
            pd2 = pd_of.pop(j)
            invt = iv_of.pop(j)
            dn = dn_pool.tile([128, 2, KT, M], BF16, tag="dn")
            nc.vector.tensor_mul(dn, pd2, invt.broadcast_to([128, 2, KT, M]))
            # t->i: per-token max over m as bf16 TT-max tree (2x mode; a
            # single TensorReduce gets no fast mode and costs ~25% more)
            src = dn
            w = M
            while w > 2:
                w //= 2
                nxt = m2_pool.tile([128, 2, KT, w], BF16, tag=f"tm{w}")
                nc.vector.tensor_max(nxt, src[:, :, :, 0:w], src[:, :, :, w:2 * w])
                src = nxt
            nc.vector.tensor_max(
                st_t_all[:, 2 * j:2 * j + 2, :].rearrange("p a (b o) -> p a b o", o=1),
                src[:, :, :, 0:1], src[:, :, :, 1:2],
            )
            # i->t: max over n, tree level 1 (8 -> 4)
            m2 = m2_pool.tile([128, 2, KT // 2, M], BF16, tag="m2")
            nc.vector.tensor_max(
                m2, dn[:, :, 0:KT // 2, :], dn[:, :, KT // 2:KT, :]
            )
            m2_of[j] = m2

        def s5(j):  # finish i->t reduction; staged sqrt every SI batches
            b0 = 2 * j
            s2i = b0 % SI
            g = b0 // SI
            if s2i == 0:
                st_i_new = stage.tile([128, SI, M], BF16, tag="sti")
                sti_of[g] = st_i_new
            st_i = sti_of[g]
            m2 = m2_of.pop(j)
            m3 = small.tile([128, 2, 2, M], BF16, tag="m3")
            nc.vector.tensor_max(m3, m2[:, :, 0:2, :], m2[:, :, 2:4, :])
            nm2 = small.tile([128, 2, M], BF16, tag="nm2")
            nc.vector.tensor_max(nm2, m3[:, :, 0, :], m3[:, :, 1, :])
            nc.gpsimd.partition_all_reduce(
                st_i[:, s2i:s2i + 2, :].rearrange("p a b -> p (a b)"),
                nm2.rearrange("p a b -> p (a b)"),
                channels=128, reduce_op=RED.max,
            )
            if s2i == SI - 2:
                del sti_of[g]
                di = stage.tile([128, SI * M], BF16, tag="di")
                nc.scalar.activation(
                    di, st_i.rearrange("p a b -> p (a b)"),
                    ACT.Sqrt, bias=two[:], scale=-2.0,
                )
                nc.gpsimd.tensor_add(acc_i, acc_i, di)

        nj = b_loc // 2
        for v in range(b_loc + 2 * LAG):
            # pair stages run at half rate, interleaved with batch stages
            if v >= LAG and (v - LAG) % 2 == 1 and (v - LAG) // 2 < nj:
                s5((v - LAG) // 2)
            if v >= 4 and (v - 4) % 2 == 1 and (v - 4) // 2 < nj:
                s4((v - 4) // 2)
            if v >= 3 and (v - 3) % 2 == 1 and (v - 3) // 2 < nj:
                s3((v - 3) // 2)
            if v >= 2 and v - 2 < b_loc:
                s2(v - 2)
            if v >= 1 and v - 1 < b_loc:
                s1(v - 1)
            if v < b_loc:
                s0(v)

        # ---------- final reductions ----------
        dt = singles.tile([128, b_loc * KT], BF16)
        nc.scalar.activation(
            dt, st_t_all.rearrange("p a b -> p (a b)"),
            ACT.Sqrt, bias=two[:], scale=-2.0,
        )
        red_t = singles.tile([128, 1], F32)
        nc.vector.tensor_reduce(red_t, dt, axis=AX.X, op=OP.add)
        rep_t = singles.tile([128, 1], F32)
        nc.gpsimd.partition_all_reduce(
            rep_t, red_t, channels=128, reduce_op=RED.add
        )
        red_i = singles.tile([128, 1], F32)
        nc.vector.tensor_reduce(red_i, acc_i, axis=AX.X, op=OP.add)
        out_sb = small.tile([1, 2], F32, tag="out_sb")
        nc.scalar.copy(out_sb[:, 0:1], rep_t[0:1, :])
        nc.scalar.copy(out_sb[:, 1:2], red_i[0:1, :])
        nc.sync.dma_start(out=out, in_=out_sb)

    nc.compile()
    return nc


_NC_CACHE = None


def _get_nc():
    global _NC_CACHE
    if _NC_CACHE is None:
        _NC_CACHE = build()
    return _NC_CACHE


def kernel(tokens: np.ndarray, interests: np.ndarray, _trace=False) -> np.ndarray:
    tokens = np.ascontiguousarray(tokens, dtype=np.float32)
    interests = np.ascontiguousarray(interests, dtype=np.float32)
    assert tokens.shape == (B, K, D) and interests.shape == (B, M, D)

    nc = _get_nc()
    in_maps = [
        {
            "tokens": tokens[c * B_LOC:(c + 1) * B_LOC],
            "interests": interests[c * B_LOC:(c + 1) * B_LOC],
        }
        for c in range(N_CORES)
    ]
    res = run_bass_kernel_spmd(
        nc, in_maps, core_ids=list(range(N_CORES)), trace=_trace
    )
    sum_t = 0.0  # sum over all (b, k) of min_m dist
    sum_i = 0.0  # sum over all (b, m) of min_k dist
    for r in res.results:
        sum_t += float(r["out"][0, 0])
        sum_i += float(r["out"][0, 1])
    loss = sum_i / (B * M) + ALPHA_T_TO_I * sum_t / (B * K)
    kernel.last_results = res
    return np.array(loss, dtype=np.float32)


# revision 15
# speedup vs baseline: 1.9715x; 1.0213x over previous
"""Chamfer loss kernel for TRN2 (8 NeuronCores, data-parallel over batch).

Reference computation (per batch b):
  t = l2_normalize(tokens[b])      # (K=1024, D=128)
  i = l2_normalize(interests[b])   # (M=64,  D=128)
  dist[k,m] = sqrt(2 - 2*dot(t_k, i_m))   (unit vectors)
  loss = mean_bm(min_k dist) + 0.3 * mean_bk(min_m dist)

Design notes (per core, 64 batches; engine-balanced against the ~1.46us/batch
token-DMA floor):
  - interests pre-normalized once (phase 0, bn_stats for sum-of-squares),
    kept transposed in SBUF as bf16 iT_all.
  - per batch:
      DMA  tokens[b] -> t_all fp32 [128,(8n),128d]
      PE   8 transposes (fp32) -> psum; ACT evacuates psum -> tT bf16
      DVE  tq = tT*tT (bf16 2x mode)
      PE   8 dot matmuls   pdots[k,(n m)] = tT_n.T @ iT_b        (bf16)
      PE   8 ones-column matmuls sums[k,n] = tq_n.T @ ones  == sum_d t^2
           (lands token sum-of-squares directly in [k-partition, n] layout:
            no partition reduce, no DMA gather)
      ACT  tnrm = sqrt(sums) from psum;  DVE invt = 1/tnrm
      DVE  dn = pdots * invt  (fused normalize + psum evacuation, bf16)
      POOL st_t_all[:,b,:] = max_m dn   (deferred sqrt, once at the end)
      POOL m2 = max(dn[:,0:4,:], dn[:,4:8,:]);  DVE m3, nmax (bf16 tree)
      POOL partition-max nmax -> st_i chunk;  every 8 batches ACT applies
           sqrt(2-2x) to the chunk and POOL accumulates.
Host combines the 8 per-core partial sums.
"""

import numpy as np
from contextlib import ExitStack

import concourse.bass as bass
import concourse.bass_isa as bass_isa
import concourse.mybir as mybir
import concourse.tile as tile
from concourse import bacc
from concourse.bass_utils import run_bass_kernel_spmd

N_CORES = 8
B, K, M, D = 512, 1024, 64, 128
B_LOC = B // N_CORES          # 64 batches per core
KT = K // 128                 # 8 token tiles of [128, D] per batch
NG = B_LOC * M // 128         # 32 interest row-groups of 128
ALPHA_T_TO_I = 0.3
SI = 8                        # i-side sqrt staging (batches per chunk)
LAG = 5

F32 = mybir.dt.float32
BF16 = mybir.dt.bfloat16
AX = mybir.AxisListType
OP = mybir.AluOpType
ACT = mybir.ActivationFunctionType
RED = bass_isa.ReduceOp


def build(b_loc=B_LOC):
    assert b_loc % SI == 0
    nc = bacc.Bacc(
        "TRN2",
        target_bir_lowering=False,
        debug=False,
        num_devices=N_CORES,
    )
    tokens = nc.dram_tensor("tokens", [b_loc, K, D], F32, kind="ExternalInput").ap()
    interests = nc.dram_tensor(
        "interests", [b_loc, M, D], F32, kind="ExternalInput"
    ).ap()
    out = nc.dram_tensor("out", [1, 2], F32, kind="ExternalOutput").ap()

    with ExitStack() as ctx:
        tc = ctx.enter_context(tile.TileContext(nc))
        singles = ctx.enter_context(tc.tile_pool(name="singles", bufs=1))
        tok_pool = ctx.enter_context(tc.tile_pool(name="tok", bufs=4))
        tT_pool = ctx.enter_context(tc.tile_pool(name="tT", bufs=4))
        tq_pool = ctx.enter_context(tc.tile_pool(name="tq", bufs=4))
        dn_pool = ctx.enter_context(tc.tile_pool(name="dn", bufs=4))
        m2_pool = ctx.enter_context(tc.tile_pool(name="m2", bufs=4))
        small = ctx.enter_context(tc.tile_pool(name="small", bufs=16))
        stage = ctx.enter_context(tc.tile_pool(name="stage", bufs=3))
        p_tT = ctx.enter_context(tc.tile_pool(name="p_tT", bufs=3, space="PSUM"))
        p_dots = ctx.enter_context(tc.tile_pool(name="p_dots", bufs=2, space="PSUM"))
        p_sums = ctx.enter_context(tc.tile_pool(name="p_sums", bufs=1, space="PSUM"))

        identity = singles.tile([128, 128], F32)
        nc.gpsimd.memset(identity, 0.0)
        nc.gpsimd.affine_select(
            out=identity, in_=identity, compare_op=OP.not_equal, fill=1.0,
            base=0, pattern=[[-1, 128]], channel_multiplier=1,
        )
        ones_bf = singles.tile([128, 1], BF16)
        nc.vector.memset(ones_bf, 1.0)
        two = singles.tile([128, 1], F32)
        nc.vector.memset(two, 2.0)
        st_t_all = singles.tile([128, b_loc, KT], BF16)
        acc_i = singles.tile([128, SI * M], F32)
        nc.vector.memset(acc_i, 0.0)

        # ---------- phase 0: normalize + transpose all interests ----------
        i_flat = interests.rearrange("b m d -> (b m) d").rearrange(
            "(g p) d -> p g d", p=128
        )  # [128, NG, 128]
        i_all = singles.tile([128, NG, D], F32)
        nc.sync.dma_start(out=i_all, in_=i_flat)

        # sum-of-squares per interest row: ACT square + DVE segmented reduce
        isq = singles.tile([128, NG, D], BF16)
        H2 = NG // 2
        nc.scalar.square(isq[:, :H2], i_all[:, :H2])
        nc.scalar.square(isq[:, H2:], i_all[:, H2:])
        issq = small.tile([128, NG], F32, tag="issq")
        nc.vector.tensor_reduce(issq, isq, axis=AX.X, op=OP.add)
        inrm = small.tile([128, NG], F32, tag="inrm")
        nc.scalar.sqrt(inrm, issq)
        invi = small.tile([128, NG], F32, tag="invi")
        nc.vector.reciprocal(invi, inrm)

        i_n = singles.tile([128, NG, D], F32)
        H = NG // 2
        nc.vector.tensor_mul(
            i_n[:, :H], i_all[:, :H], invi[:, :H].broadcast_to([128, H, D])
        )
        nc.gpsimd.tensor_mul(
            i_n[:, H:], i_all[:, H:], invi[:, H:].broadcast_to([128, H, D])
        )

        iT_all = singles.tile([128, NG, 128], BF16)  # [d, (g, row)]
        for c in range(0, NG, 4):
            piT = p_tT.tile([128, 4, 128], F32, tag="ptT")
            for j in range(4):
                nc.tensor.transpose(
                    piT[:, j, :], i_n[:, c + j, :], identity
                )
            dst = iT_all[:, c:c + 4, :].rearrange("p a b -> p (a b)")
            src = piT[:, :4, :].rearrange("p a b -> p (a b)")
            if c < 24:
                nc.scalar.copy(dst, src)
            else:
                nc.vector.tensor_copy(dst, src)

        def iT_of(b):
            return iT_all[:, b // 2, (b % 2) * M:(b % 2) * M + M]

        # ---------- software-pipelined main loop ----------
        # Post-matmul vector work is fused over batch PAIRS to amortize the
        # fixed per-op access latencies on DVE.  All free-axis reductions and
        # maxes are DVE-only (gpsimd has neither); Pool gets the elementwise
        # square's other half, the partition-max, and the accumulate adds.
        t_of, tT_of, pd_of, ps_of, iv_of = {}, {}, {}, {}, {}
        m2_of, sti_of = {}, {}

        def s0(b):  # token DMA
            t_all = tok_pool.tile([128, KT, D], F32)
            nc.sync.dma_start(
                out=t_all, in_=tokens[b].rearrange("(n p) d -> p n d", p=128)
            )
            t_of[b] = t_all

        def s1(b):  # transposes + evacuation (fp32 psum -> bf16 sbuf)
            t_all = t_of.pop(b)
            tT = tT_pool.tile([128, KT, 128], BF16, tag="tT")
            for h in range(2):
                ptT = p_tT.tile([128, KT // 2, 128], F32, tag="ptT")
                for j in range(KT // 2):
                    nc.tensor.transpose(
                        ptT[:, j, :], t_all[:, 4 * h + j, :], identity
                    )
                nc.scalar.copy(
                    tT[:, 4 * h:4 * h + 4, :].rearrange("p a b -> p (a b)"),
                    ptT.rearrange("p a b -> p (a b)"),
                )
            tT_of[b] = tT

        def s2(b):  # squares (DVE/Pool halves), dots, sum-of-squares columns
            tT = tT_of.pop(b)
            tq = tq_pool.tile([128, KT, 128], BF16, tag="tq")
            nc.vector.tensor_mul(
                tq[:, 0:4, :].rearrange("p a b -> p (a b)"),
                tT[:, 0:4, :].rearrange("p a b -> p (a b)"),
                tT[:, 0:4, :].rearrange("p a b -> p (a b)"),
            )
            nc.gpsimd.tensor_mul(
                tq[:, 4:8, :].rearrange("p a b -> p (a b)"),
                tT[:, 4:8, :].rearrange("p a b -> p (a b)"),
                tT[:, 4:8, :].rearrange("p a b -> p (a b)"),
            )
            if b % 2 == 0:
                pd2 = p_dots.tile([128, 2, KT, M], F32, tag="pd")
                ps2 = p_sums.tile([128, 2, KT], F32, tag="ps")
                pd_of[b // 2] = pd2
                ps_of[b // 2] = ps2
            else:
                pd2 = pd_of[b // 2]
                ps2 = ps_of[b // 2]
            h = b % 2
            iT = iT_of(b)
            for n in range(KT):
                nc.tensor.matmul(
                    ps2[:, h, n:n + 1], lhsT=tq[:, n, :], rhs=ones_bf,
                    start=True, stop=True,
                )
            for n in range(KT):
                nc.tensor.matmul(
                    pd2[:, h, n, :], lhsT=tT[:, n, :], rhs=iT,
                    start=True, stop=True,
                )

        def s3(j):  # token norms for pair j
            tnrm = small.tile([128, 2, KT], F32, tag="tnrm")
            nc.scalar.sqrt(tnrm, ps_of.pop(j))
            invt = small.tile([128, 2, KT], F32, tag="invt")
            nc.vector.reciprocal(invt, tnrm)
            iv_of[j] = invt

        def s4(j):  # normalize; max over m; first n-tree level  (pair j)
            pd2 = pd_of.pop(j)
            invt = iv_of.pop(j)
            dn = dn_pool.tile([128, 2, KT, M], BF16, tag="dn")
            nc.vector.tensor_mul(dn, pd2, invt.broadcast_to([128, 2, KT, M]))
            # t->i: per-token max over m as bf16 TT-max tree (2x mode; a
            # single TensorReduce gets no fast mode and costs ~25% more)
            src = dn
            w = M
            while w > 2:
                w //= 2
                nxt = m2_pool.tile([128, 2, KT, w], BF16, tag=f"tm{w}")
                nc.vector.tensor_max(nxt, src[:, :, :, 0:w], src[:, :, :, w:2 * w])
                src = nxt
            nc.vector.tensor_max(
                st_t_all[:, 2 * j:2 * j + 2, :].rearrange("p a (b o) -> p a b o", o=1),
                src[:, :, :, 0:1], src[:, :, :, 1:2],
            )
            # i->t: max over n, tree level 1 (8 -> 4)
            m2 = m2_pool.tile([128, 2, KT // 2, M], BF16, tag="m2")
            nc.vector.tensor_max(
                m2, dn[:, :, 0:KT // 2, :], dn[:, :, KT // 2:KT, :]
            )
            m2_of[j] = m2

        def s5(j):  # finish i->t reduction; staged sqrt every SI batches
            b0 = 2 * j
            s2i = b0 % SI
            g = b0 // SI
            if s2i == 0:
                st_i_new = stage.tile([128, SI, M], BF16, tag="sti")
                sti_of[g] = st_i_new
            st_i = sti_of[g]
            m2 = m2_of.pop(j)
            m3 = small.tile([128, 2, 2, M], BF16, tag="m3")
            nc.vector.tensor_max(m3, m2[:, :, 0:2, :], m2[:, :, 2:4, :])
            nm2 = small.tile([128, 2, M], BF16, tag="nm2")
            nc.vector.tensor_max(nm2, m3[:, :, 0, :], m3[:, :, 1, :])
            nc.gpsimd.partition_all_reduce(
                st_i[:, s2i:s2i + 2, :].rearrange("p a b -> p (a b)"),
                nm2.rearrange("p a b -> p (a b)"),
                channels=128, reduce_op=RED.max,
            )
            if s2i == SI - 2:
                del sti_of[g]
                di = stage.tile([128, SI * M], BF16, tag="di")
                nc.scalar.activation(
                    di, st_i.rearrange("p a b -> p (a b)"),
                    ACT.Sqrt, bias=two[:], scale=-2.0,
                )
                nc.gpsimd.tensor_add(acc_i, acc_i, di)

        nj = b_loc // 2
        for v in range(b_loc + 2 * LAG):
            # pair stages run at half rate, interleaved with batch stages
            if v >= LAG and (v - LAG) % 2 == 1 and (v - LAG) // 2 < nj:
                s5((v - LAG) // 2)
            if v >= 4 and (v - 4) % 2 == 1 and (v - 4) // 2 < nj:
                s4((v - 4) // 2)
            if v >= 3 and (v - 3) % 2 == 1 and (v - 3) // 2 < nj:
                s3((v - 3) // 2)
            if v >= 2 and v - 2 < b_loc:
                s2(v - 2)
            if v >= 1 and v - 1 < b_loc:
                s1(v - 1)
            if v < b_loc:
                s0(v)

        # ---------- final reductions ----------
        dt = singles.tile([128, b_loc * KT], BF16)
        nc.scalar.activation(
            dt, st_t_all.rearrange("p a b -> p (a b)"),
            ACT.Sqrt, bias=two[:], scale=-2.0,
        )
        red_t = singles.tile([128, 1], F32)
        nc.vector.tensor_reduce(red_t, dt, axis=AX.X, op=OP.add)
        rep_t = singles.tile([128, 1], F32)
        nc.gpsimd.partition_all_reduce(
            rep_t, red_t, channels=128, reduce_op=RED.add
        )
        red_i = singles.tile([128, 1], F32)
        nc.vector.tensor_reduce(red_i, acc_i, axis=AX.X, op=OP.add)
        out_sb = small.tile([1, 2], F32, tag="out_sb")
        nc.scalar.copy(out_sb[:, 0:1], rep_t[0:1, :])
        nc.scalar.copy(out_sb[:, 1:2], red_i[0:1, :])
        nc.sync.dma_start(out=out, in_=out_sb)

    nc.compile()
    return nc


_NC_CACHE = None


def _get_nc():
    global _NC_CACHE
    if _NC_CACHE is None:
        _NC_CACHE = build()
    return _NC_CACHE


def kernel(tokens: np.ndarray, interests: np.ndarray, _trace=False) -> np.ndarray:
    tokens = np.ascontiguousarray(tokens, dtype=np.float32)
    interests = np.ascontiguousarray(interests, dtype=np.float32)
    assert tokens.shape == (B, K, D) and interests.shape == (B, M, D)

    nc = _get_nc()
    in_maps = [
        {
            "tokens": tokens[c * B_LOC:(c + 1) * B_LOC],
            "interests": interests[c * B_LOC:(c + 1) * B_LOC],
        }
        for c in range(N_CORES)
    ]
    res = run_bass_kernel_spmd(
        nc, in_maps, core_ids=list(range(N_CORES)), trace=_trace
    )
    sum_t = 0.0  # sum over all (b, k) of min_m dist
    sum_i = 0.0  # sum over all (b, m) of min_k dist
    for r in res.results:
        sum_t += float(r["out"][0, 0])
        sum_i += float(r["out"][0, 1])
    loss = sum_i / (B * M) + ALPHA_T_TO_I * sum_t / (B * K)
    kernel.last_results = res
    return np.array(loss, dtype=np.float32)


# revision 17
# speedup vs baseline: 2.0533x; 1.0415x over previous
"""Chamfer loss kernel for TRN2 (8 NeuronCores, data-parallel over batch).

Reference computation (per batch b):
  t = l2_normalize(tokens[b])      # (K=1024, D=128)
  i = l2_normalize(interests[b])   # (M=64,  D=128)
  dist[k,m] = sqrt(2 - 2*dot(t_k, i_m))   (unit vectors)
  loss = mean_bm(min_k dist) + 0.3 * mean_bk(min_m dist)

Design notes (per core, 64 batches; engine-balanced against the ~1.46us/batch
token-DMA floor):
  - interests pre-normalized once (phase 0, bn_stats for sum-of-squares),
    kept transposed in SBUF as bf16 iT_all.
  - per batch:
      DMA  tokens[b] -> t_all fp32 [128,(8n),128d]
      PE   8 transposes (fp32) -> psum; ACT evacuates psum -> tT bf16
      DVE  tq = tT*tT (bf16 2x mode)
      PE   8 dot matmuls   pdots[k,(n m)] = tT_n.T @ iT_b        (bf16)
      PE   8 ones-column matmuls sums[k,n] = tq_n.T @ ones  == sum_d t^2
           (lands token sum-of-squares directly in [k-partition, n] layout:
            no partition reduce, no DMA gather)
      ACT  tnrm = sqrt(sums) from psum;  DVE invt = 1/tnrm
      DVE  dn = pdots * invt  (fused normalize + psum evacuation, bf16)
      POOL st_t_all[:,b,:] = max_m dn   (deferred sqrt, once at the end)
      POOL m2 = max(dn[:,0:4,:], dn[:,4:8,:]);  DVE m3, nmax (bf16 tree)
      POOL partition-max nmax -> st_i chunk;  every 8 batches ACT applies
           sqrt(2-2x) to the chunk and POOL accumulates.
Host combines the 8 per-core partial sums.
"""

import numpy as np
from contextlib import ExitStack

import concourse.bass as bass
import concourse.bass_isa as bass_isa
import concourse.mybir as mybir
import concourse.tile as tile
from concourse import bacc
from concourse.bass_utils import run_bass_kernel_spmd

N_CORES = 8
B, K, M, D = 512, 1024, 64, 128
B_LOC = B // N_CORES          # 64 batches per core
KT = K // 128                 # 8 token tiles of [128, D] per batch
NG = B_LOC * M // 128         # 32 interest row-groups of 128
ALPHA_T_TO_I = 0.3
SI = 8                        # i-side sqrt staging (batches per chunk)
LAG = 5

F32 = mybir.dt.float32
BF16 = mybir.dt.bfloat16
AX = mybir.AxisListType
OP = mybir.AluOpType
ACT = mybir.ActivationFunctionType
RED = bass_isa.ReduceOp


def build(b_loc=B_LOC):
    assert b_loc % SI == 0
    nc = bacc.Bacc(
        "TRN2",
        target_bir_lowering=False,
        debug=False,
        num_devices=N_CORES,
    )
    tokens = nc.dram_tensor("tokens", [b_loc, K, D], F32, kind="ExternalInput").ap()
    interests = nc.dram_tensor(
        "interests", [b_loc, M, D], F32, kind="ExternalInput"
    ).ap()
    out = nc.dram_tensor("out", [1, 2], F32, kind="ExternalOutput").ap()

    with ExitStack() as ctx:
        tc = ctx.enter_context(tile.TileContext(nc))
        singles = ctx.enter_context(tc.tile_pool(name="singles", bufs=1))
        tok_pool = ctx.enter_context(tc.tile_pool(name="tok", bufs=4))
        tT_pool = ctx.enter_context(tc.tile_pool(name="tT", bufs=4))
        tq_pool = ctx.enter_context(tc.tile_pool(name="tq", bufs=4))
        dn_pool = ctx.enter_context(tc.tile_pool(name="dn", bufs=4))
        m2_pool = ctx.enter_context(tc.tile_pool(name="m2", bufs=4))
        small = ctx.enter_context(tc.tile_pool(name="small", bufs=16))
        stage = ctx.enter_context(tc.tile_pool(name="stage", bufs=3))
        p_tT = ctx.enter_context(tc.tile_pool(name="p_tT", bufs=3, space="PSUM"))
        p_dots = ctx.enter_context(tc.tile_pool(name="p_dots", bufs=2, space="PSUM"))
        p_sums = ctx.enter_context(tc.tile_pool(name="p_sums", bufs=1, space="PSUM"))

        identity = singles.tile([128, 128], F32)
        nc.gpsimd.memset(identity, 0.0)
        nc.gpsimd.affine_select(
            out=identity, in_=identity, compare_op=OP.not_equal, fill=1.0,
            base=0, pattern=[[-1, 128]], channel_multiplier=1,
        )
        ones_bf = singles.tile([128, 1], BF16)
        nc.vector.memset(ones_bf, 1.0)
        two = singles.tile([128, 1], F32)
        nc.vector.memset(two, 2.0)
        st_t_all = singles.tile([128, b_loc, KT], BF16)
        acc_i = singles.tile([128, SI * M], F32)
        nc.vector.memset(acc_i, 0.0)

        # ---------- phase 0: normalize + transpose all interests ----------
        # Emitted in 4 chunks, interleaved into the main pipeline so the
        # first token batches' DMA/transpose/square stages are not serialized
        # behind the whole interests preparation.
        i_flat = interests.rearrange("b m d -> (b m) d").rearrange(
            "(g p) d -> p g d", p=128
        )  # [128, NG, 128]
        i_all = singles.tile([128, NG, D], F32)
        i_n = singles.tile([128, NG, D], F32)
        iT_all = singles.tile([128, NG, 128], BF16)  # [d, (g, row)]
        NCH = 4
        GC = NG // NCH  # 8 groups per chunk

        def pA(c):  # interests chunk DMA
            sl = slice(GC * c, GC * (c + 1))
            nc.sync.dma_start(out=i_all[:, sl], in_=i_flat[:, sl])

        def pB(c):  # sum-of-squares per interest row
            sl = slice(GC * c, GC * (c + 1))
            isq = tq_pool.tile([128, GC, D], BF16, tag="isq")
            nc.scalar.square(isq, i_all[:, sl])
            issq = small.tile([128, GC], F32, tag="issq")
            nc.vector.tensor_reduce(issq, isq, axis=AX.X, op=OP.add)
            inrm = small.tile([128, GC], F32, tag="inrm")
            nc.scalar.sqrt(inrm, issq)
            invi = small.tile([128, GC], F32, tag="invi")
            nc.vector.reciprocal(invi, inrm)
            nc.gpsimd.tensor_mul(
                i_n[:, sl], i_all[:, sl], invi.broadcast_to([128, GC, D])
            )

        def pC(c):  # transpose + evacuate chunk
            for cc in range(GC * c, GC * (c + 1), 4):
                piT = p_tT.tile([128, 4, 128], F32, tag="ptT")
                for j in range(4):
                    nc.tensor.transpose(
                        piT[:, j, :], i_n[:, cc + j, :], identity
                    )
                dst = iT_all[:, cc:cc + 4, :].rearrange("p a b -> p (a b)")
                src = piT[:, :4, :].rearrange("p a b -> p (a b)")
                nc.scalar.copy(dst, src)

        def iT_of(b):
            return iT_all[:, b // 2, (b % 2) * M:(b % 2) * M + M]

        # ---------- software-pipelined main loop ----------
        # Post-matmul vector work is fused over batch PAIRS to amortize the
        # fixed per-op access latencies on DVE.  All free-axis reductions and
        # maxes are DVE-only (gpsimd has neither); Pool gets the elementwise
        # square's other half, the partition-max, and the accumulate adds.
        t_of, tT_of, pd_of, ps_of, iv_of = {}, {}, {}, {}, {}
        m2_of, sti_of = {}, {}

        def s0(b):  # token DMA
            t_all = tok_pool.tile([128, KT, D], F32)
            nc.sync.dma_start(
                out=t_all, in_=tokens[b].rearrange("(n p) d -> p n d", p=128)
            )
            t_of[b] = t_all

        def s1(b):  # transposes + evacuation (fp32 psum -> bf16 sbuf)
            t_all = t_of.pop(b)
            tT = tT_pool.tile([128, KT, 128], BF16, tag="tT")
            for h in range(2):
                ptT = p_tT.tile([128, KT // 2, 128], F32, tag="ptT")
                for j in range(KT // 2):
                    nc.tensor.transpose(
                        ptT[:, j, :], t_all[:, 4 * h + j, :], identity
                    )
                nc.scalar.copy(
                    tT[:, 4 * h:4 * h + 4, :].rearrange("p a b -> p (a b)"),
                    ptT.rearrange("p a b -> p (a b)"),
                )
            tT_of[b] = tT

        def s2(b):  # squares (DVE/Pool halves), dots, sum-of-squares columns
            tT = tT_of.pop(b)
            tq = tq_pool.tile([128, KT, 128], BF16, tag="tq")
            nc.vector.tensor_mul(
                tq[:, 0:4, :].rearrange("p a b -> p (a b)"),
                tT[:, 0:4, :].rearrange("p a b -> p (a b)"),
                tT[:, 0:4, :].rearrange("p a b -> p (a b)"),
            )
            nc.gpsimd.tensor_mul(
                tq[:, 4:8, :].rearrange("p a b -> p (a b)"),
                tT[:, 4:8, :].rearrange("p a b -> p (a b)"),
                tT[:, 4:8, :].rearrange("p a b -> p (a b)"),
            )
            if b % 2 == 0:
                pd2 = p_dots.tile([128, 2, KT, M], F32, tag="pd")
                ps2 = p_sums.tile([128, 2, KT], F32, tag="ps")
                pd_of[b // 2] = pd2
                ps_of[b // 2] = ps2
            else:
                pd2 = pd_of[b // 2]
                ps2 = ps_of[b // 2]
            h = b % 2
            iT = iT_of(b)
            for n in range(KT):
                nc.tensor.matmul(
                    ps2[:, h, n:n + 1], lhsT=tq[:, n, :], rhs=ones_bf,
                    start=True, stop=True,
                )
            for n in range(KT):
                nc.tensor.matmul(
                    pd2[:, h, n, :], lhsT=tT[:, n, :], rhs=iT,
                    start=True, stop=True,
                )

        def s3(j):  # token norms for pair j
            tnrm = small.tile([128, 2, KT], F32, tag="tnrm")
            nc.scalar.sqrt(tnrm, ps_of.pop(j))
            invt = small.tile([128, 2, KT], F32, tag="invt")
            nc.vector.reciprocal(invt, tnrm)
            iv_of[j] = invt

        q5_of, m2q_of = {}, {}

        def s4(j):  # normalize; m-tree levels 1-2; n-tree level 1  (pair j)
            pd2 = pd_of.pop(j)
            invt = iv_of.pop(j)
            q = j // 2
            h = j % 2
            if h == 0:
                q5_new = m2_pool.tile([128, 2, 2, KT, 16], BF16, tag="q5")
                m2q_new = m2_pool.tile([128, 2, 2, KT // 2, M], BF16, tag="m2q")
                q5_of[q] = q5_new
                m2q_of[q] = m2q_new
            dn = dn_pool.tile([128, 2, KT, M], BF16, tag="dn")
            nc.vector.tensor_mul(dn, pd2, invt.broadcast_to([128, 2, KT, M]))
            # t->i: per-token max over m, bf16 TT-max tree levels 64->32->16
            t32 = m2_pool.tile([128, 2, KT, 32], BF16, tag="t32")
            nc.vector.tensor_max(t32, dn[:, :, :, 0:32], dn[:, :, :, 32:64])
            nc.vector.tensor_max(
                q5_of[q][:, h], t32[:, :, :, 0:16], t32[:, :, :, 16:32]
            )
            # i->t: max over n, tree level 1 (8 -> 4)
            nc.vector.tensor_max(
                m2q_of[q][:, h], dn[:, :, 0:KT // 2, :], dn[:, :, KT // 2:KT, :]
            )

        def s5(q):  # finish both reductions for quad q (4 batches)
            b0 = 4 * q
            s2i = b0 % SI
            g = b0 // SI
            if s2i == 0:
                st_i_new = stage.tile([128, SI, M], BF16, tag="sti")
                sti_of[g] = st_i_new
            st_i = sti_of[g]
            # t->i tail: 16 -> 8 -> 4 -> 2 -> 1 over m
            q5 = q5_of.pop(q)
            t8 = small.tile([128, 2, 2, KT, 8], BF16, tag="t8")
            nc.vector.tensor_max(t8, q5[:, :, :, :, 0:8], q5[:, :, :, :, 8:16])
            t4 = small.tile([128, 2, 2, KT, 4], BF16, tag="t4")
            nc.vector.tensor_max(t4, t8[:, :, :, :, 0:4], t8[:, :, :, :, 4:8])
            t2 = small.tile([128, 2, 2, KT, 2], BF16, tag="t2")
            nc.vector.tensor_max(t2, t4[:, :, :, :, 0:2], t4[:, :, :, :, 2:4])
            nc.vector.tensor_max(
                st_t_all[:, b0:b0 + 4, :].rearrange(
                    "p (a b) (c o) -> p a b c o", a=2, o=1
                ),
                t2[:, :, :, :, 0:1], t2[:, :, :, :, 1:2],
            )
            # i->t tail: n-tree levels 2-3, then partition max
            m2q = m2q_of.pop(q)
            m3 = small.tile([128, 2, 2, 2, M], BF16, tag="m3")
            nc.vector.tensor_max(m3, m2q[:, :, :, 0:2, :], m2q[:, :, :, 2:4, :])
            nm2 = small.tile([128, 2, 2, M], BF16, tag="nm2")
            nc.vector.tensor_max(nm2, m3[:, :, :, 0, :], m3[:, :, :, 1, :])
            nc.gpsimd.partition_all_reduce(
                st_i[:, s2i:s2i + 4, :].rearrange("p a b -> p (a b)"),
                nm2.rearrange("p a b c -> p (a b c)"),
                channels=128, reduce_op=RED.max,
            )
            if s2i == SI - 4:
                del sti_of[g]
                di = stage.tile([128, SI * M], BF16, tag="di")
                nc.scalar.activation(
                    di, st_i.rearrange("p a b -> p (a b)"),
                    ACT.Sqrt, bias=two[:], scale=-2.0,
                )
                nc.gpsimd.tensor_add(acc_i, acc_i, di)

        nj = b_loc // 2
        nq = b_loc // 4
        for v in range(b_loc + 2 * LAG + 2):
            # interleaved phase-0 chunks
            if v % 2 == 0 and v // 2 < NCH:
                pA(v // 2)
            if v % 2 == 1 and v // 2 < NCH:
                pB(v // 2)
            if v >= 2 and v % 2 == 0 and (v - 2) // 2 < NCH:
                pC((v - 2) // 2)
            # quad stages
            if v >= 8 and (v - 8) % 4 == 0 and (v - 8) // 4 < nq:
                s5((v - 8) // 4)
            # pair stages
            if v >= 4 and (v - 4) % 2 == 1 and (v - 4) // 2 < nj:
                s4((v - 4) // 2)
            if v >= 3 and (v - 3) % 2 == 1 and (v - 3) // 2 < nj:
                s3((v - 3) // 2)
            # batch stages
            if v >= 2 and v - 2 < b_loc:
                s2(v - 2)
            if v >= 1 and v - 1 < b_loc:
                s1(v - 1)
            if v < b_loc:
                s0(v)

        # ---------- final reductions ----------
        dt = singles.tile([128, b_loc * KT], BF16)
        nc.scalar.activation(
            dt, st_t_all.rearrange("p a b -> p (a b)"),
            ACT.Sqrt, bias=two[:], scale=-2.0,
        )
        red_t = singles.tile([128, 1], F32)
        nc.vector.tensor_reduce(red_t, dt, axis=AX.X, op=OP.add)
        rep_t = singles.tile([128, 1], F32)
        nc.gpsimd.partition_all_reduce(
            rep_t, red_t, channels=128, reduce_op=RED.add
        )
        red_i = singles.tile([128, 1], F32)
        nc.vector.tensor_reduce(red_i, acc_i, axis=AX.X, op=OP.add)
        out_sb = small.tile([1, 2], F32, tag="out_sb")
        nc.scalar.copy(out_sb[:, 0:1], rep_t[0:1, :])
        nc.scalar.copy(out_sb[:, 1:2], red_i[0:1, :])
        nc.sync.dma_start(out=out, in_=out_sb)

    nc.compile()
    return nc


_NC_CACHE = None


def _get_nc():
    global _NC_CACHE
    if _NC_CACHE is None:
        _NC_CACHE = build()
    return _NC_CACHE


def kernel(tokens: np.ndarray, interests: np.ndarray, _trace=False) -> np.ndarray:
    tokens = np.ascontiguousarray(tokens, dtype=np.float32)
    interests = np.ascontiguousarray(interests, dtype=np.float32)
    assert tokens.shape == (B, K, D) and interests.shape == (B, M, D)

    nc = _get_nc()
    in_maps = [
        {
            "tokens": tokens[c * B_LOC:(c + 1) * B_LOC],
            "interests": interests[c * B_LOC:(c + 1) * B_LOC],
        }
        for c in range(N_CORES)
    ]
    res = run_bass_kernel_spmd(
        nc, in_maps, core_ids=list(range(N_CORES)), trace=_trace
    )
    sum_t = 0.0  # sum over all (b, k) of min_m dist
    sum_i = 0.0  # sum over all (b, m) of min_k dist
    for r in res.results:
        sum_t += float(r["out"][0, 0])
        sum_i += float(r["out"][0, 1])
    loss = sum_i / (B * M) + ALPHA_T_TO_I * sum_t / (B * K)
    kernel.last_results = res
    return np.array(loss, dtype=np.float32)
